# revision 1
# baseline (speedup 1.0000x reference)
"""Trainium2 Bass kernel: EMA along L + residual for x of shape (32, 4096, 512).

reference semantics (fp32):
    s_0 = x_0 ; s_t = 0.3*x_t + 0.7*s_{t-1}   (per (b, d), scan along L)
    returns (x - s, s)

Implementation: with beta = 0.7, the scan is a causal exponential filter whose
taps decay below 1e-20 after 128 steps, so each 128-row output block is (to
well below fp32 resolution) an exact linear function of the current and
previous 128-row input blocks:

    y[j, d] = sum_k Wp[k, j] * x_prev[k, d] + sum_k Wc[k, j] * x_cur[k, d]

with Wc[k, j] = 0.3 * 0.7^(j-k) (j >= k), Wp[k, j] = 0.3 * 0.7^(j+128-k),
and a special first-block matrix whose k=0 row carries x_0's coefficient
0.7^j. That turns the sequential scan into independent 128x128xD matmuls on
the PE array -- no carry chain at all.

Sharding: batch dim (32) split 4-per-core across 8 NeuronCores; the scan dim
L stays on-core so there is no cross-device communication.
"""

import sys

import numpy as np

try:
    import concourse.bass as bass  # noqa: F401
except ImportError:  # container puts the repo at /opt/trn_rl_repo
    sys.path.insert(0, "/opt/trn_rl_repo")

import concourse.bacc as bacc
import concourse.bass as bass
import concourse.mybir as mybir
import concourse.tile as tile
from concourse.bass_utils import run_bass_kernel_spmd

ALPHA = 0.3
BETA = 0.7

B, L, D = 32, 4096, 512
NCORES = 8
BLOC = B // NCORES  # batches per core
PB = 128  # L-block (partition dim of matmul inputs)
NBLK = L // PB  # 32 L-blocks per batch
G = 8  # L-blocks per DMA granule (8 * 128 rows * 512 d * 4B = 2 MiB)
NGRAN = NBLK // G

_F32 = mybir.dt.float32


def _weights():
    n = np.arange(PB, dtype=np.float64)
    jk = n[None, :] - n[:, None]  # j - k
    wc = np.where(jk >= 0, ALPHA * BETA ** np.clip(jk, 0, None), 0.0)
    wcf = wc.copy()
    wcf[0, :] = BETA**n  # first block: x_0 enters with coefficient 0.7^j
    wp = ALPHA * BETA ** (n[None, :] + PB - n[:, None])
    out = []
    for w in (wc, wcf, wp):
        w = w.astype(np.float32)
        w[np.abs(w) < 1e-35] = 0.0  # avoid fp32 denormals in the PE
        out.append(np.ascontiguousarray(w))
    return out


_NC_CACHE = None


def build():
    """Build + compile the per-core Bass program (identical on all 8 cores)."""
    global _NC_CACHE
    if _NC_CACHE is not None:
        return _NC_CACHE

    nc = bacc.Bacc("TRN2", target_bir_lowering=False, debug=False, num_devices=NCORES)

    x_d = nc.dram_tensor("x_shard", [BLOC, L, D], _F32, kind="ExternalInput")
    ma_d = nc.dram_tensor("ma_shard", [BLOC, L, D], _F32, kind="ExternalOutput")
    res_d = nc.dram_tensor("res_shard", [BLOC, L, D], _F32, kind="ExternalOutput")

    wc_np, wcf_np, wp_np = _weights()
    wc_d = nc.inline_tensor(wc_np, name="wc_const")
    wcf_d = nc.inline_tensor(wcf_np, name="wcf_const")
    wp_d = nc.inline_tensor(wp_np, name="wp_const")

    xa, maa, ra = x_d.ap(), ma_d.ap(), res_d.ap()

    with tile.TileContext(nc) as tc:
        with (
            tc.tile_pool(name="consts", bufs=1) as consts,
            tc.tile_pool(name="xpool", bufs=3) as xpool,
            tc.tile_pool(name="mapool", bufs=3) as mapool,
            tc.tile_pool(name="respool", bufs=3) as respool,
            tc.tile_pool(name="psum", bufs=8, space=bass.MemorySpace.PSUM) as psum,
        ):
            wc_s = consts.tile([PB, PB], _F32, tag="wc")
            nc.sync.dma_start(wc_s[:], wc_d.ap())
            wcf_s = consts.tile([PB, PB], _F32, tag="wcf")
            nc.sync.dma_start(wcf_s[:], wcf_d.ap())
            wp_s = consts.tile([PB, PB], _F32, tag="wp")
            nc.sync.dma_start(wp_s[:], wp_d.ap())

            for b in range(BLOC):
                xg_prev = None
                for g in range(NGRAN):
                    l0 = g * G * PB
                    src = xa[b, l0 : l0 + G * PB, :].rearrange(
                        "(g p) d -> p g d", p=PB
                    )
                    xg = xpool.tile([PB, G, D], _F32, tag="xg")
                    nc.sync.dma_start(xg[:], src)

                    mag = mapool.tile([PB, G, D], _F32, tag="mag")
                    resg = respool.tile([PB, G, D], _F32, tag="resg")

                    for i in range(G):
                        yt = psum.tile([PB, D], _F32, tag="yt")
                        if g == 0 and i == 0:
                            # first L-block of this batch: single matmul with
                            # the special first-block weights
                            nc.tensor.matmul(
                                yt[:], wcf_s[:], xg[:, 0, :], start=True, stop=True
                            )
                        else:
                            xprev = xg[:, i - 1, :] if i > 0 else xg_prev[:, G - 1, :]
                            nc.tensor.matmul(
                                yt[:], wp_s[:], xprev, start=True, stop=False
                            )
                            nc.tensor.matmul(
                                yt[:], wc_s[:], xg[:, i, :], start=False, stop=True
                            )
                        # ma tile to SBUF on the scalar engine
                        nc.scalar.copy(mag[:, i, :], yt[:])
                        # res = x - ma on the vector engine (PSUM operand ok)
                        nc.vector.tensor_sub(resg[:, i, :], xg[:, i, :], yt[:])

                    dst_ma = maa[b, l0 : l0 + G * PB, :].rearrange(
                        "(g p) d -> p g d", p=PB
                    )
                    dst_res = ra[b, l0 : l0 + G * PB, :].rearrange(
                        "(g p) d -> p g d", p=PB
                    )
                    nc.sync.dma_start(dst_ma, mag[:])
                    nc.sync.dma_start(dst_res, resg[:])
                    xg_prev = xg

    nc.compile()
    _NC_CACHE = nc
    return nc


def kernel(**inputs):
    x = np.ascontiguousarray(inputs["x"], dtype=np.float32)
    assert x.shape == (B, L, D), x.shape

    nc = build()
    in_maps = [{"x_shard": x[c * BLOC : (c + 1) * BLOC]} for c in range(NCORES)]
    r = run_bass_kernel_spmd(nc, in_maps, core_ids=list(range(NCORES)))

    res = np.concatenate([r.results[c]["res_shard"] for c in range(NCORES)], axis=0)
    ma = np.concatenate([r.results[c]["ma_shard"] for c in range(NCORES)], axis=0)
    return (res, ma)


if __name__ == "__main__":
    rng = np.random.default_rng(0)
    x = rng.standard_normal((B, L, D)).astype(np.float32)
    res, ma = kernel(x=x)
    print("ok", res.shape, ma.shape, float(np.abs(ma).max()))
